# revision 10
# baseline (speedup 1.0000x reference)
"""DPGGAN forward on 8 Trainium2 NeuronCores (Bass/Tile, SPMD).

Sharding: data-parallel over the B=4096 seed nodes (512 per core).
Each core gathers its shard's neighbor features (the dominant memory
traffic), runs encoder+decoder in transposed activation layout
([feature, seed]), AllGathers the normalized emb2 and the per-node
discriminator projection v, then computes its row-slice of
reconst_adj and the two discriminator probes.

The per-core work is split into two seed-halves so the first half's
decoder + AllGather + reconst overlap with the second half's gathers.

Algebraic simplifications (exact up to fp rounding):
 - l2norm row-scales cancel through subsequent l2norms
   (l2norm(c*x)=l2norm(x), relu(c*x)=c*relu(x) for c>0), so only the
   emb1/emb2 norms are computed.
 - (X @ xw + bg) @ Wdisc = X @ (F @ Wg @ Wdisc) + bg @ Wdisc, collapsing
   the BxB discriminator matmuls to matvecs with v = F @ (Wg @ Wdisc).
 - einsum/S and mean-over-S fold into W1 (x0.1), W2 (x0.1), W3 (x-0.1).
"""
import numpy as np

B = 4096
S = 10
NCORES = 8
BLOC = B // NCORES            # 512 seeds per core
NLOC = BLOC * S               # 5120 layer-1 nodes per core
NT = NLOC // 128              # 40 gather tiles
GRP = 5                       # tiles per seed-aligned reduce group (640 nodes)
QN = 4
QB = BLOC // QN               # 128 seeds per quarter
FEAT = 256
L2 = 128
V = 100000
P = 128

_COMPILED = None
TRACE = False
LAST_EXEC_NS = None
LAST_RES = None


def _build():
    from concourse import bass, bacc, mybir, tile
    from concourse.masks import make_identity

    f32 = mybir.dt.float32
    i32 = mybir.dt.int32
    ACT = mybir.ActivationFunctionType

    nc = bacc.Bacc("TRN2", target_bir_lowering=False, debug=False,
                   enable_asserts=True, num_devices=NCORES)

    t_feat = nc.dram_tensor("feat", [V, FEAT], f32, kind="ExternalInput")
    t_nidx = nc.dram_tensor("nidx", [NLOC, S], i32, kind="ExternalInput")
    t_nodes = nc.dram_tensor("nodes", [BLOC], i32, kind="ExternalInput")
    t_sub = nc.dram_tensor("subadj", [BLOC, B], f32, kind="ExternalInput")
    t_W1 = nc.dram_tensor("W1", [FEAT, FEAT], f32, kind="ExternalInput")
    t_W2 = nc.dram_tensor("W2", [FEAT, L2], f32, kind="ExternalInput")
    t_W3 = nc.dram_tensor("W3", [FEAT, L2], f32, kind="ExternalInput")
    t_Wd1 = nc.dram_tensor("Wd1", [L2, FEAT], f32, kind="ExternalInput")
    t_Wd2 = nc.dram_tensor("Wd2", [FEAT, FEAT], f32, kind="ExternalInput")
    t_Wm1 = nc.dram_tensor("Wm1", [FEAT, FEAT], f32, kind="ExternalInput")
    t_Wm2 = nc.dram_tensor("Wm2", [FEAT, FEAT], f32, kind="ExternalInput")
    t_Wg = nc.dram_tensor("Wg", [FEAT, FEAT], f32, kind="ExternalInput")
    t_bg = nc.dram_tensor("bg", [FEAT], f32, kind="ExternalInput")
    t_Wdisc = nc.dram_tensor("Wdisc", [FEAT, 1], f32, kind="ExternalInput")

    o_mu = nc.dram_tensor("mu", [BLOC, L2], f32, kind="ExternalOutput")
    o_lv = nc.dram_tensor("lv", [BLOC, L2], f32, kind="ExternalOutput")
    o_rec = nc.dram_tensor("rec", [BLOC, B], f32, kind="ExternalOutput")
    o_orig = nc.dram_tensor("orig", [BLOC, 1], f32, kind="ExternalOutput")
    o_gen = nc.dram_tensor("gen", [BLOC, 1], f32, kind="ExternalOutput")

    with tile.TileContext(nc) as tc:
        with tc.tile_pool(name="const", bufs=1) as const, \
             tc.tile_pool(name="gath", bufs=4) as gath, \
             tc.tile_pool(name="work", bufs=2) as work, \
             tc.tile_pool(name="dec", bufs=6) as decp, \
             tc.tile_pool(name="persist", bufs=1) as persist, \
             tc.tile_pool(name="sub", bufs=1) as subp, \
             tc.tile_pool(name="psA", bufs=6, space="PSUM") as psA, \
             tc.tile_pool(name="psS", bufs=2, space="PSUM") as psS, \
             tc.tile_pool(name="dram", bufs=1, space="DRAM") as dramp:

            ident = const.tile([P, P], f32)
            make_identity(nc, ident[:])
            ones = const.tile([P, 1], f32)
            nc.vector.memset(ones[:], 1.0)

            # ---------- weights ----------
            def load_scaled(dram_ap, halves, width, scale, nm):
                raw = work.tile([P, halves, width], f32, name=nm + "r",
                                tag="wraw")
                nc.sync.dma_start(
                    out=raw[:],
                    in_=dram_ap.rearrange("(a p) d -> p a d", p=P))
                dst = const.tile([P, halves, width], f32, name=nm)
                nc.scalar.activation(out=dst[:], in_=raw[:], func=ACT.Copy,
                                     scale=scale)
                return dst

            W1e = load_scaled(t_W1.ap(), 2, FEAT, 0.1, "W1e")
            W2e = load_scaled(t_W2.ap(), 2, L2, 0.1, "W2e")
            W3e = load_scaled(t_W3.ap(), 2, L2, -0.1, "W3e")
            Wd1s = const.tile([P, FEAT], f32)
            nc.sync.dma_start(out=Wd1s[:], in_=t_Wd1.ap())
            Wd2s = const.tile([P, 2, FEAT], f32)
            nc.sync.dma_start(out=Wd2s[:],
                              in_=t_Wd2.ap().rearrange("(a p) d -> p a d", p=P))
            Wm1s = const.tile([P, 2, FEAT], f32)
            nc.sync.dma_start(out=Wm1s[:],
                              in_=t_Wm1.ap().rearrange("(a p) d -> p a d", p=P))
            Wm2s = const.tile([P, 2, FEAT], f32)
            nc.sync.dma_start(out=Wm2s[:],
                              in_=t_Wm2.ap().rearrange("(a p) d -> p a d", p=P))
            Wgs = const.tile([P, 2, FEAT], f32)
            nc.sync.dma_start(out=Wgs[:],
                              in_=t_Wg.ap().rearrange("(a p) d -> p a d", p=P))
            bg_row = const.tile([1, FEAT], f32)
            nc.sync.dma_start(out=bg_row[:], in_=t_bg.ap()[None, :])
            wdisc_row = const.tile([1, FEAT], f32)
            nc.sync.dma_start(out=wdisc_row[:],
                              in_=t_Wdisc.ap().rearrange("d one -> one d"))

            # ---------- index tiles ----------
            idx_all = const.tile([P, NT, S], i32)
            nc.sync.dma_start(
                out=idx_all[:],
                in_=t_nidx.ap().rearrange("(t p) s -> p t s", p=P))
            nodes_idx = const.tile([P, 4], i32)
            nc.sync.dma_start(out=nodes_idx[:],
                              in_=t_nodes.ap().rearrange("(m p) -> p m", p=P))

            # ---------- discriminator prep: u, F, v (early) ----------
            ub_w = const.tile([P, FEAT], f32)
            nc.gpsimd.partition_broadcast(ub_w[:], wdisc_row[:])
            u_half = work.tile([P, 2], f32, name="u_half")
            for i in range(2):
                tw = work.tile([P, FEAT], f32, name="tw", tag="tw")
                nc.vector.tensor_mul(out=tw[:], in0=Wgs[:, i, :], in1=ub_w[:])
                nc.vector.reduce_sum(out=u_half[:, i:i + 1], in_=tw[:],
                                     axis=mybir.AxisListType.X)
            u_d = dramp.tile([2, P], f32)
            for i in range(2):
                nc.sync.dma_start(out=u_d[i, :], in_=u_half[:, i:i + 1])
            u_row = work.tile([1, FEAT], f32, name="u_row")
            nc.sync.dma_start(out=u_row[:],
                              in_=u_d[:].rearrange("a p -> (a p)")[None, :])
            ub = const.tile([P, FEAT], f32)
            nc.gpsimd.partition_broadcast(ub[:], u_row[:])

            v_d = dramp.tile([4, P], f32)
            for m in range(4):
                F = gath.tile([P, FEAT], f32, name="F", tag="F")
                nc.gpsimd.indirect_dma_start(
                    out=F[:], out_offset=None, in_=t_feat.ap(),
                    in_offset=bass.IndirectOffsetOnAxis(
                        ap=nodes_idx[:, m:m + 1], axis=0))
                tv = work.tile([P, FEAT], f32, name="tv", tag="tv")
                nc.vector.tensor_mul(out=tv[:], in0=F[:], in1=ub[:])
                vm = work.tile([P, 1], f32, name="vm", tag="vm")
                nc.vector.reduce_sum(out=vm[:], in_=tv[:],
                                     axis=mybir.AxisListType.X)
                nc.sync.dma_start(out=v_d[m, :], in_=vm[:])
            v_all = dramp.tile([NCORES, 4 * P], f32, addr_space="Shared")
            nc.gpsimd.collective_compute(
                "AllGather", mybir.AluOpType.bypass,
                replica_groups=[list(range(NCORES))],
                ins=[v_d[:]], outs=[v_all[:]])
            vb = persist.tile([P, B], f32)
            v_flat = v_all[:].rearrange("c n -> (c n)")
            nc.sync.dma_start(out=vb[:],
                              in_=v_flat[None, :].to_broadcast([P, B]))

            tb = work.tile([1, FEAT], f32, name="tb")
            nc.vector.tensor_mul(out=tb[:], in0=bg_row[:], in1=wdisc_row[:])
            c_sb = work.tile([1, 1], f32, name="c_sb")
            nc.vector.reduce_sum(out=c_sb[:], in_=tb[:],
                                 axis=mybir.AxisListType.X)
            cb = const.tile([P, 1], f32)
            nc.gpsimd.partition_broadcast(cb[:], c_sb[:])

            # ---------- orig = sub_adj @ v + c (early, hides under gather) --
            for mm in range(4):
                oacc = work.tile([P, 1], f32, name="oacc", tag="oacc")
                for h2 in range(2):
                    sub_t = subp.tile([P, B // 2], f32, name="sub_t",
                                      tag="subt")
                    nc.sync.dma_start(
                        out=sub_t[:],
                        in_=t_sub.ap()[mm * P:(mm + 1) * P,
                                       h2 * (B // 2):(h2 + 1) * (B // 2)])
                    for rr in range(4):
                        r_ = h2 * 4 + rr
                        ot = work.tile([P, BLOC], f32, name="ot", tag="ot")
                        nc.vector.tensor_mul(
                            out=ot[:],
                            in0=sub_t[:, rr * BLOC:(rr + 1) * BLOC],
                            in1=vb[:, r_ * BLOC:(r_ + 1) * BLOC])
                        ors = work.tile([P, 1], f32, name="ors", tag="ors")
                        nc.vector.reduce_sum(out=ors[:], in_=ot[:],
                                             axis=mybir.AxisListType.X)
                        if r_ == 0:
                            nc.vector.tensor_copy(out=oacc[:], in_=ors[:])
                        else:
                            nc.vector.tensor_add(out=oacc[:], in0=oacc[:],
                                                 in1=ors[:])
                osb2 = work.tile([P, 1], f32, name="osb2", tag="osb2")
                nc.vector.tensor_add(out=osb2[:], in0=oacc[:], in1=cb[:])
                nc.sync.dma_start(out=o_orig.ap()[mm * P:(mm + 1) * P, :],
                                  in_=osb2[:])

            # ---------- persistent tiles ----------
            aggT2 = [persist.tile([P, BLOC], f32, name=f"aggT2{m}")
                     for m in range(2)]
            muT = persist.tile([P, BLOC], f32)
            lvT = persist.tile([P, BLOC], f32)
            e1nT = [persist.tile([P, BLOC], f32, name=f"e1nT{m}")
                    for m in range(2)]
            ones1 = const.tile([1, P], f32)
            nc.vector.memset(ones1[:], 1.0)
            cc_in = [dramp.tile([2 * P, QB], f32, name=f"ccin{q}")
                     for q in range(QN)]
            cc_out = [dramp.tile([NCORES * 2 * P, QB], f32,
                                 addr_space="Shared", name=f"ccout{q}")
                      for q in range(QN)]
            R = [[[None] * QN for _ in range(2)] for _ in range(NCORES)]
            w_part = [[None] * 2 for _ in range(QN)]

            def emit_tiles(t0, t1):
                for t in range(t0, t1):
                    if t % GRP == 0:
                        emit_tiles.h1win = [
                            work.tile([P, GRP * P], f32, name=f"h1w{m}",
                                      tag=f"h1w{m}") for m in range(2)]
                    h1win = emit_tiles.h1win
                    G = gath.tile([P, S, FEAT], f32, name="G", tag="G")
                    for s in range(S):
                        nc.gpsimd.indirect_dma_start(
                            out=G[:, s, :], out_offset=None, in_=t_feat.ap(),
                            in_offset=bass.IndirectOffsetOnAxis(
                                ap=idx_all[:, t, s:s + 1], axis=0))
                    nc.vector.tensor_add(out=G[:, 0:5, :], in0=G[:, 0:5, :],
                                         in1=G[:, 5:10, :])
                    nc.vector.tensor_add(out=G[:, 0:2, :], in0=G[:, 0:2, :],
                                         in1=G[:, 2:4, :])
                    nc.vector.tensor_add(out=G[:, 0, :], in0=G[:, 0, :],
                                         in1=G[:, 1, :])
                    nc.vector.tensor_add(out=G[:, 0, :], in0=G[:, 0, :],
                                         in1=G[:, 4, :])
                    aggT = work.tile([P, 2, P], f32, name="aggT", tag="aggT")
                    for k in range(2):
                        trp = psA.tile([P, P], f32, space="PSUM", tag="ps")
                        nc.tensor.matmul(out=trp[:],
                                         lhsT=G[:, 0, k * P:(k + 1) * P],
                                         rhs=ident[:], is_transpose=True)
                        nc.scalar.activation(out=aggT[:, k, :], in_=trp[:],
                                             func=ACT.Copy)
                    for m in range(2):
                        h1p = psA.tile([P, P], f32, space="PSUM", tag="ps")
                        for k in range(2):
                            nc.tensor.matmul(
                                out=h1p[:],
                                lhsT=W1e[:, k, m * P:(m + 1) * P],
                                rhs=aggT[:, k, :],
                                start=(k == 0), stop=(k == 1))
                        nc.scalar.activation(
                            out=h1win[m][:, (t % GRP) * P:(t % GRP + 1) * P],
                            in_=h1p[:], func=ACT.Relu)
                    if t % GRP == GRP - 1:
                        g = t // GRP
                        for m in range(2):
                            seg = h1win[m][:].rearrange("p (b s) -> p b s",
                                                        s=S)
                            nc.vector.reduce_sum(
                                out=aggT2[m][:, g * 64:(g + 1) * 64],
                                in_=seg, axis=mybir.AxisListType.X)

            def dec_compute(q):
                qs = slice(q * QB, (q + 1) * QB)
                for (wt, dst) in ((W2e, muT), (W3e, lvT)):
                    mp = psA.tile([P, QB], f32, space="PSUM", tag="ps")
                    for k in range(2):
                        nc.tensor.matmul(out=mp[:], lhsT=wt[:, k, :],
                                         rhs=aggT2[k][:, qs],
                                         start=(k == 0), stop=(k == 1))
                    nc.scalar.activation(out=dst[:, qs], in_=mp[:],
                                         func=ACT.Copy)
                for (srcT, dst) in ((muT, o_mu), (lvT, o_lv)):
                    trp = psA.tile([P, P], f32, space="PSUM", tag="ps")
                    nc.tensor.matmul(out=trp[:], lhsT=srcT[:, qs],
                                     rhs=ident[:], is_transpose=True)
                    osb = work.tile([P, P], f32, name="osb", tag="osb")
                    nc.scalar.activation(out=osb[:], in_=trp[:],
                                         func=ACT.Copy)
                    nc.sync.dma_start(out=dst.ap()[q * P:(q + 1) * P, :],
                                      in_=osb[:])

                o1T = []
                for m in range(2):
                    xp = psA.tile([P, QB], f32, space="PSUM", tag="ps")
                    nc.tensor.matmul(out=xp[:],
                                     lhsT=Wd1s[:, m * P:(m + 1) * P],
                                     rhs=muT[:, qs], start=True, stop=True)
                    o1 = decp.tile([P, QB], f32, name=f"o1T{m}", tag="dec")
                    nc.scalar.activation(out=o1[:], in_=xp[:], func=ACT.Relu)
                    o1T.append(o1)
                o2T = []
                for m in range(2):
                    xp = psA.tile([P, QB], f32, space="PSUM", tag="ps")
                    for k in range(2):
                        nc.tensor.matmul(out=xp[:],
                                         lhsT=Wd2s[:, k, m * P:(m + 1) * P],
                                         rhs=o1T[k][:],
                                         start=(k == 0), stop=(k == 1))
                    o2 = decp.tile([P, QB], f32, name=f"o2T{m}", tag="dec")
                    nc.scalar.activation(out=o2[:], in_=xp[:], func=ACT.Relu)
                    o2T.append(o2)

                def emb(wt, nm):
                    tiles = []
                    for m in range(2):
                        xp = psA.tile([P, QB], f32, space="PSUM", tag="ps")
                        for k in range(2):
                            nc.tensor.matmul(
                                out=xp[:],
                                lhsT=wt[:, k, m * P:(m + 1) * P],
                                rhs=o2T[k][:],
                                start=(k == 0), stop=(k == 1))
                        e = decp.tile([P, QB], f32, name=f"{nm}{m}",
                                      tag="dec")
                        nc.scalar.activation(out=e[:], in_=xp[:],
                                             func=ACT.Copy)
                        tiles.append(e)
                    return tiles

                E1T = emb(Wm1s, "E1T")
                E2T = emb(Wm2s, "E2T")

                def colnorm_scale(Es, outs):
                    ssp = psS.tile([1, QB], f32, space="PSUM", tag="small")
                    sqs = []
                    for m in range(2):
                        sq = work.tile([P, QB], f32, name=f"sq{m}",
                                       tag=f"sq{m}")
                        nc.vector.tensor_mul(out=sq[:], in0=Es[m][:],
                                             in1=Es[m][:])
                        sqs.append(sq)
                    for m in range(2):
                        nc.tensor.matmul(out=ssp[:], lhsT=ones[:],
                                         rhs=sqs[m][:],
                                         start=(m == 0), stop=(m == 1))
                    r = work.tile([1, QB], f32, name="r", tag="r")
                    nc.scalar.activation(out=r[:], in_=ssp[:], func=ACT.Sqrt)
                    nc.vector.tensor_scalar_max(out=r[:], in0=r[:],
                                                scalar1=1e-12)
                    nc.vector.reciprocal(out=r[:], in_=r[:])
                    # broadcast r along partitions on the PE (K=1 matmul)
                    bc = psA.tile([P, QB], f32, space="PSUM", tag="ps")
                    nc.tensor.matmul(out=bc[:], lhsT=ones1[:], rhs=r[:],
                                     start=True, stop=True)
                    for m in range(2):
                        nc.vector.tensor_mul(out=outs[m], in0=Es[m][:],
                                             in1=bc[:])

                qs2 = slice(q * QB, (q + 1) * QB)
                e2h = [decp.tile([P, QB], f32, name=f"e2h{m}", tag="dec")
                       for m in range(2)]
                colnorm_scale(E1T, [e1nT[m][:, qs2] for m in range(2)])
                colnorm_scale(E2T, [e2h[m][:] for m in range(2)])
                for m in range(2):
                    nc.sync.dma_start(out=cc_in[q][m * P:(m + 1) * P, :],
                                      in_=e2h[m][:])

            def ag_and_w(q):
                nc.gpsimd.collective_compute(
                    "AllGather", mybir.AluOpType.bypass,
                    replica_groups=[list(range(NCORES))],
                    ins=[cc_in[q][:]], outs=[cc_out[q][:]])
                for r_ in range(NCORES):
                    for k in range(2):
                        Rt = persist.tile([P, QB], f32, name=f"R{r_}_{k}_{q}")
                        nc.sync.dma_start(
                            out=Rt[:],
                            in_=cc_out[q][r_ * 2 * P + k * P:
                                          r_ * 2 * P + (k + 1) * P, :])
                        R[r_][k][q] = Rt
                for k in range(2):
                    wacc = persist.tile([P, 1], f32, name=f"wacc{q}{k}")
                    for r_ in range(NCORES):
                        tt = work.tile([P, QB], f32, name="tt", tag="tt")
                        nc.vector.tensor_mul(
                            out=tt[:], in0=R[r_][k][q][:],
                            in1=vb[:, r_ * BLOC + q * QB:
                                   r_ * BLOC + (q + 1) * QB])
                        rs = work.tile([P, 1], f32, name="rs", tag="rs")
                        nc.vector.reduce_sum(out=rs[:], in_=tt[:],
                                             axis=mybir.AxisListType.X)
                        if r_ == 0:
                            nc.vector.tensor_copy(out=wacc[:], in_=rs[:])
                        else:
                            nc.vector.tensor_add(out=wacc[:], in0=wacc[:],
                                                 in1=rs[:])
                    w_part[q][k] = wacc

            def rec_pair(rr, qc):
                # rows: m-tile rr; cols: quarter qc of every rank
                for r_ in range(NCORES):
                    rp = psA.tile([P, QB], f32, space="PSUM", tag="ps")
                    for k in range(2):
                        nc.tensor.matmul(
                            out=rp[:],
                            lhsT=e1nT[k][:, rr * P:(rr + 1) * P],
                            rhs=R[r_][k][qc][:],
                            start=(k == 0), stop=(k == 1))
                    rsb = work.tile([P, QB], f32, name="rsb", tag="rsb")
                    nc.scalar.activation(out=rsb[:], in_=rp[:], func=ACT.Copy)
                    nc.sync.dma_start(
                        out=o_rec.ap()[rr * P:(rr + 1) * P,
                                       r_ * BLOC + qc * QB:
                                       r_ * BLOC + (qc + 1) * QB],
                        in_=rsb[:])

            # ---------- main schedule ----------
            # quarter q tiles: [10q, 10q+10). dec_compute(q) right after its
            # tiles; AG(q) trigger a full quarter later so its inputs are
            # ready when the gpsimd stream reaches it (no gather stall).
            emit_tiles(0, 10)
            emit_tiles(10, 20)
            dec_compute(0)
            emit_tiles(20, 30)
            ag_and_w(0)
            dec_compute(1)
            emit_tiles(30, 40)
            ag_and_w(1)
            dec_compute(2)
            rec_pair(0, 0)
            ag_and_w(2)
            dec_compute(3)
            for (rr, qc) in ((0, 1), (1, 1), (1, 0)):
                rec_pair(rr, qc)
            ag_and_w(3)
            for (rr, qc) in ((0, 2), (1, 2), (2, 2), (2, 0), (2, 1)):
                rec_pair(rr, qc)
            for (rr, qc) in ((0, 3), (1, 3), (2, 3), (3, 3), (3, 0), (3, 1),
                             (3, 2)):
                rec_pair(rr, qc)

            # ---------- gen = e1n @ w + c ----------
            w_k = []
            for k in range(2):
                wk = work.tile([P, 1], f32, name=f"wk{k}", tag="wk")
                nc.vector.tensor_add(out=wk[:], in0=w_part[0][k][:],
                                     in1=w_part[1][k][:])
                nc.vector.tensor_add(out=wk[:], in0=wk[:],
                                     in1=w_part[2][k][:])
                nc.vector.tensor_add(out=wk[:], in0=wk[:],
                                     in1=w_part[3][k][:])
                w_k.append(wk)
            for mm in range(4):
                gp = psS.tile([P, 1], f32, space="PSUM", tag="small")
                for k in range(2):
                    nc.tensor.matmul(out=gp[:],
                                     lhsT=e1nT[k][:, mm * P:(mm + 1) * P],
                                     rhs=w_k[k][:],
                                     start=(k == 0), stop=(k == 1))
                gsb = work.tile([P, 1], f32, name="gsb", tag="gsb")
                nc.vector.tensor_add(out=gsb[:], in0=gp[:], in1=cb[:])
                nc.sync.dma_start(out=o_gen.ap()[mm * P:(mm + 1) * P, :],
                                  in_=gsb[:])

    nc.compile()
    return nc


def _get_compiled():
    global _COMPILED
    if _COMPILED is None:
        _COMPILED = _build()
    return _COMPILED


def kernel(nodes, neighs2, sub_adj, feat_table, W1, W2, W3,
           Wd1, Wd2, Wm1, Wm2, Wg, bg, Wdisc):
    global LAST_EXEC_NS, LAST_RES
    from concourse.bass_utils import run_bass_kernel_spmd

    nodes = np.asarray(nodes).astype(np.int32)
    neighs2 = np.asarray(neighs2).astype(np.int32)
    sub_adj = np.ascontiguousarray(np.asarray(sub_adj, dtype=np.float32))
    feat_table = np.ascontiguousarray(np.asarray(feat_table, dtype=np.float32))
    ws = {}
    for nm, w in (("W1", W1), ("W2", W2), ("W3", W3), ("Wd1", Wd1),
                  ("Wd2", Wd2), ("Wm1", Wm1), ("Wm2", Wm2), ("Wg", Wg),
                  ("bg", bg), ("Wdisc", Wdisc)):
        ws[nm] = np.ascontiguousarray(np.asarray(w, dtype=np.float32))

    nc = _get_compiled()

    in_maps = []
    for c in range(NCORES):
        m = {
            "feat": feat_table,
            "nidx": np.ascontiguousarray(neighs2[c * NLOC:(c + 1) * NLOC, :]),
            "nodes": np.ascontiguousarray(nodes[c * BLOC:(c + 1) * BLOC]),
            "subadj": np.ascontiguousarray(sub_adj[c * BLOC:(c + 1) * BLOC, :]),
        }
        m.update(ws)
        in_maps.append(m)

    res = run_bass_kernel_spmd(nc, in_maps, core_ids=list(range(NCORES)),
                               trace=TRACE)
    LAST_EXEC_NS = res.exec_time_ns
    LAST_RES = res

    mu = np.concatenate([res.results[c]["mu"] for c in range(NCORES)], axis=0)
    lv = np.concatenate([res.results[c]["lv"] for c in range(NCORES)], axis=0)
    rec = np.concatenate([res.results[c]["rec"] for c in range(NCORES)],
                         axis=0)
    orig = np.concatenate([res.results[c]["orig"] for c in range(NCORES)],
                          axis=0)
    gen = np.concatenate([res.results[c]["gen"] for c in range(NCORES)],
                         axis=0)
    gan_pred = np.concatenate([orig, gen], axis=0)
    gan_label = np.concatenate(
        [np.ones_like(orig), np.zeros_like(gen)], axis=0)
    return (mu, lv, rec, gan_pred, gan_label)


# revision 11
# speedup vs baseline: 1.1367x; 1.1367x over previous
"""DPGGAN forward on 8 Trainium2 NeuronCores (Bass/Tile, SPMD).

Sharding: data-parallel over the B=4096 seed nodes (512 per core).
Each core gathers its shard's neighbor features (the dominant memory
traffic), runs encoder+decoder in transposed activation layout
([feature, seed]), AllGathers the normalized emb2 and the per-node
discriminator projection v, then computes its row-slice of
reconst_adj and the two discriminator probes.

The per-core work is split into two seed-halves so the first half's
decoder + AllGather + reconst overlap with the second half's gathers.

Algebraic simplifications (exact up to fp rounding):
 - l2norm row-scales cancel through subsequent l2norms
   (l2norm(c*x)=l2norm(x), relu(c*x)=c*relu(x) for c>0), so only the
   emb1/emb2 norms are computed.
 - (X @ xw + bg) @ Wdisc = X @ (F @ Wg @ Wdisc) + bg @ Wdisc, collapsing
   the BxB discriminator matmuls to matvecs with v = F @ (Wg @ Wdisc).
 - einsum/S and mean-over-S fold into W1 (x0.1), W2 (x0.1), W3 (x-0.1).
"""
import numpy as np

B = 4096
S = 10
NCORES = 8
BLOC = B // NCORES            # 512 seeds per core
NLOC = BLOC * S               # 5120 layer-1 nodes per core
NT = NLOC // 128              # 40 gather tiles
GRP = 5                       # tiles per seed-aligned reduce group (640 nodes)
QN = 2
QB = BLOC // QN               # 256 seeds per half-shard
FEAT = 256
L2 = 128
V = 100000
P = 128

_COMPILED = None
TRACE = False
LAST_EXEC_NS = None
LAST_RES = None


def _build():
    from concourse import bass, bacc, mybir, tile
    from concourse.masks import make_identity

    f32 = mybir.dt.float32
    i32 = mybir.dt.int32
    ACT = mybir.ActivationFunctionType

    nc = bacc.Bacc("TRN2", target_bir_lowering=False, debug=False,
                   enable_asserts=True, num_devices=NCORES)

    t_feat = nc.dram_tensor("feat", [V, FEAT], f32, kind="ExternalInput")
    t_nidx = nc.dram_tensor("nidx", [NLOC, S], i32, kind="ExternalInput")
    t_nodes = nc.dram_tensor("nodes", [BLOC], i32, kind="ExternalInput")
    t_sub = nc.dram_tensor("subadj", [BLOC, B], f32, kind="ExternalInput")
    t_W1 = nc.dram_tensor("W1", [FEAT, FEAT], f32, kind="ExternalInput")
    t_W2 = nc.dram_tensor("W2", [FEAT, L2], f32, kind="ExternalInput")
    t_W3 = nc.dram_tensor("W3", [FEAT, L2], f32, kind="ExternalInput")
    t_Wd1 = nc.dram_tensor("Wd1", [L2, FEAT], f32, kind="ExternalInput")
    t_Wd2 = nc.dram_tensor("Wd2", [FEAT, FEAT], f32, kind="ExternalInput")
    t_Wm1 = nc.dram_tensor("Wm1", [FEAT, FEAT], f32, kind="ExternalInput")
    t_Wm2 = nc.dram_tensor("Wm2", [FEAT, FEAT], f32, kind="ExternalInput")
    t_Wg = nc.dram_tensor("Wg", [FEAT, FEAT], f32, kind="ExternalInput")
    t_bg = nc.dram_tensor("bg", [FEAT], f32, kind="ExternalInput")
    t_Wdisc = nc.dram_tensor("Wdisc", [FEAT, 1], f32, kind="ExternalInput")

    o_mu = nc.dram_tensor("mu", [BLOC, L2], f32, kind="ExternalOutput")
    o_lv = nc.dram_tensor("lv", [BLOC, L2], f32, kind="ExternalOutput")
    o_rec = nc.dram_tensor("rec", [BLOC, B], f32, kind="ExternalOutput")
    o_orig = nc.dram_tensor("orig", [BLOC, 1], f32, kind="ExternalOutput")
    o_gen = nc.dram_tensor("gen", [BLOC, 1], f32, kind="ExternalOutput")

    with tile.TileContext(nc) as tc:
        with tc.tile_pool(name="const", bufs=1) as const, \
             tc.tile_pool(name="gath", bufs=4) as gath, \
             tc.tile_pool(name="work", bufs=2) as work, \
             tc.tile_pool(name="dec", bufs=6) as decp, \
             tc.tile_pool(name="persist", bufs=1) as persist, \
             tc.tile_pool(name="sub", bufs=1) as subp, \
             tc.tile_pool(name="psA", bufs=6, space="PSUM") as psA, \
             tc.tile_pool(name="psS", bufs=2, space="PSUM") as psS, \
             tc.tile_pool(name="dram", bufs=1, space="DRAM") as dramp:

            ident = const.tile([P, P], f32)
            make_identity(nc, ident[:])
            ones = const.tile([P, 1], f32)
            nc.vector.memset(ones[:], 1.0)

            # ---------- weights ----------
            def load_scaled(dram_ap, halves, width, scale, nm):
                raw = work.tile([P, halves, width], f32, name=nm + "r",
                                tag="wraw")
                nc.sync.dma_start(
                    out=raw[:],
                    in_=dram_ap.rearrange("(a p) d -> p a d", p=P))
                dst = const.tile([P, halves, width], f32, name=nm)
                nc.scalar.activation(out=dst[:], in_=raw[:], func=ACT.Copy,
                                     scale=scale)
                return dst

            W1e = load_scaled(t_W1.ap(), 2, FEAT, 0.1, "W1e")
            W2e = load_scaled(t_W2.ap(), 2, L2, 0.1, "W2e")
            W3e = load_scaled(t_W3.ap(), 2, L2, -0.1, "W3e")
            Wd1s = const.tile([P, FEAT], f32)
            nc.sync.dma_start(out=Wd1s[:], in_=t_Wd1.ap())
            Wd2s = const.tile([P, 2, FEAT], f32)
            nc.sync.dma_start(out=Wd2s[:],
                              in_=t_Wd2.ap().rearrange("(a p) d -> p a d", p=P))
            Wm1s = const.tile([P, 2, FEAT], f32)
            nc.sync.dma_start(out=Wm1s[:],
                              in_=t_Wm1.ap().rearrange("(a p) d -> p a d", p=P))
            Wm2s = const.tile([P, 2, FEAT], f32)
            nc.sync.dma_start(out=Wm2s[:],
                              in_=t_Wm2.ap().rearrange("(a p) d -> p a d", p=P))
            Wgs = const.tile([P, 2, FEAT], f32)
            nc.sync.dma_start(out=Wgs[:],
                              in_=t_Wg.ap().rearrange("(a p) d -> p a d", p=P))
            bg_row = const.tile([1, FEAT], f32)
            nc.sync.dma_start(out=bg_row[:], in_=t_bg.ap()[None, :])
            wdisc_row = const.tile([1, FEAT], f32)
            nc.sync.dma_start(out=wdisc_row[:],
                              in_=t_Wdisc.ap().rearrange("d one -> one d"))

            # ---------- index tiles ----------
            idx_all = const.tile([P, NT, S], i32)
            nc.sync.dma_start(
                out=idx_all[:],
                in_=t_nidx.ap().rearrange("(t p) s -> p t s", p=P))
            nodes_idx = const.tile([P, 4], i32)
            nc.sync.dma_start(out=nodes_idx[:],
                              in_=t_nodes.ap().rearrange("(m p) -> p m", p=P))

            # ---------- discriminator prep: u, F, v (early) ----------
            ub_w = const.tile([P, FEAT], f32)
            nc.gpsimd.partition_broadcast(ub_w[:], wdisc_row[:])
            u_half = work.tile([P, 2], f32, name="u_half")
            for i in range(2):
                tw = work.tile([P, FEAT], f32, name="tw", tag="tw")
                nc.vector.tensor_mul(out=tw[:], in0=Wgs[:, i, :], in1=ub_w[:])
                nc.vector.reduce_sum(out=u_half[:, i:i + 1], in_=tw[:],
                                     axis=mybir.AxisListType.X)
            u_d = dramp.tile([2, P], f32)
            for i in range(2):
                nc.sync.dma_start(out=u_d[i, :], in_=u_half[:, i:i + 1])
            u_row = work.tile([1, FEAT], f32, name="u_row")
            nc.sync.dma_start(out=u_row[:],
                              in_=u_d[:].rearrange("a p -> (a p)")[None, :])
            ub = const.tile([P, FEAT], f32)
            nc.gpsimd.partition_broadcast(ub[:], u_row[:])

            v_d = dramp.tile([4, P], f32)
            for m in range(4):
                F = gath.tile([P, FEAT], f32, name="F", tag="F")
                nc.gpsimd.indirect_dma_start(
                    out=F[:], out_offset=None, in_=t_feat.ap(),
                    in_offset=bass.IndirectOffsetOnAxis(
                        ap=nodes_idx[:, m:m + 1], axis=0))
                tv = work.tile([P, FEAT], f32, name="tv", tag="tv")
                nc.vector.tensor_mul(out=tv[:], in0=F[:], in1=ub[:])
                vm = work.tile([P, 1], f32, name="vm", tag="vm")
                nc.vector.reduce_sum(out=vm[:], in_=tv[:],
                                     axis=mybir.AxisListType.X)
                nc.sync.dma_start(out=v_d[m, :], in_=vm[:])
            v_all = dramp.tile([NCORES, 4 * P], f32, addr_space="Shared")
            nc.gpsimd.collective_compute(
                "AllGather", mybir.AluOpType.bypass,
                replica_groups=[list(range(NCORES))],
                ins=[v_d[:]], outs=[v_all[:]])
            vb = persist.tile([P, B], f32)
            v_flat = v_all[:].rearrange("c n -> (c n)")
            nc.sync.dma_start(out=vb[:],
                              in_=v_flat[None, :].to_broadcast([P, B]))

            tb = work.tile([1, FEAT], f32, name="tb")
            nc.vector.tensor_mul(out=tb[:], in0=bg_row[:], in1=wdisc_row[:])
            c_sb = work.tile([1, 1], f32, name="c_sb")
            nc.vector.reduce_sum(out=c_sb[:], in_=tb[:],
                                 axis=mybir.AxisListType.X)
            cb = const.tile([P, 1], f32)
            nc.gpsimd.partition_broadcast(cb[:], c_sb[:])

            # ---------- orig = sub_adj @ v + c (early, hides under gather) --
            for mm in range(4):
                oacc = work.tile([P, 1], f32, name="oacc", tag="oacc")
                for h2 in range(2):
                    sub_t = subp.tile([P, B // 2], f32, name="sub_t",
                                      tag="subt")
                    nc.sync.dma_start(
                        out=sub_t[:],
                        in_=t_sub.ap()[mm * P:(mm + 1) * P,
                                       h2 * (B // 2):(h2 + 1) * (B // 2)])
                    for rr in range(4):
                        r_ = h2 * 4 + rr
                        ot = work.tile([P, BLOC], f32, name="ot", tag="ot")
                        nc.vector.tensor_mul(
                            out=ot[:],
                            in0=sub_t[:, rr * BLOC:(rr + 1) * BLOC],
                            in1=vb[:, r_ * BLOC:(r_ + 1) * BLOC])
                        ors = work.tile([P, 1], f32, name="ors", tag="ors")
                        nc.vector.reduce_sum(out=ors[:], in_=ot[:],
                                             axis=mybir.AxisListType.X)
                        if r_ == 0:
                            nc.vector.tensor_copy(out=oacc[:], in_=ors[:])
                        else:
                            nc.vector.tensor_add(out=oacc[:], in0=oacc[:],
                                                 in1=ors[:])
                osb2 = work.tile([P, 1], f32, name="osb2", tag="osb2")
                nc.vector.tensor_add(out=osb2[:], in0=oacc[:], in1=cb[:])
                nc.sync.dma_start(out=o_orig.ap()[mm * P:(mm + 1) * P, :],
                                  in_=osb2[:])

            # ---------- persistent tiles ----------
            aggT2 = [persist.tile([P, BLOC], f32, name=f"aggT2{m}")
                     for m in range(2)]
            muT = persist.tile([P, BLOC], f32)
            lvT = persist.tile([P, BLOC], f32)
            e1nT = [persist.tile([P, BLOC], f32, name=f"e1nT{m}")
                    for m in range(2)]
            ones1 = const.tile([1, P], f32)
            nc.vector.memset(ones1[:], 1.0)
            cc_in = [dramp.tile([2 * P, QB], f32, name=f"ccin{q}")
                     for q in range(QN)]
            cc_out = [dramp.tile([NCORES * 2 * P, QB], f32,
                                 addr_space="Shared", name=f"ccout{q}")
                      for q in range(QN)]
            R = [[[None] * QN for _ in range(2)] for _ in range(NCORES)]
            w_part = [[None] * 2 for _ in range(QN)]

            def emit_tiles(t0, t1):
                for t in range(t0, t1):
                    if t % GRP == 0:
                        emit_tiles.h1win = [
                            work.tile([P, GRP * P], f32, name=f"h1w{m}",
                                      tag=f"h1w{m}") for m in range(2)]
                    h1win = emit_tiles.h1win
                    G = gath.tile([P, S, FEAT], f32, name="G", tag="G")
                    for s in range(S):
                        nc.gpsimd.indirect_dma_start(
                            out=G[:, s, :], out_offset=None, in_=t_feat.ap(),
                            in_offset=bass.IndirectOffsetOnAxis(
                                ap=idx_all[:, t, s:s + 1], axis=0))
                    nc.vector.tensor_add(out=G[:, 0:5, :], in0=G[:, 0:5, :],
                                         in1=G[:, 5:10, :])
                    nc.vector.tensor_add(out=G[:, 0:2, :], in0=G[:, 0:2, :],
                                         in1=G[:, 2:4, :])
                    nc.vector.tensor_add(out=G[:, 0, :], in0=G[:, 0, :],
                                         in1=G[:, 1, :])
                    nc.vector.tensor_add(out=G[:, 0, :], in0=G[:, 0, :],
                                         in1=G[:, 4, :])
                    aggT = work.tile([P, 2, P], f32, name="aggT", tag="aggT")
                    for k in range(2):
                        trp = psA.tile([P, P], f32, space="PSUM", tag="ps")
                        nc.tensor.matmul(out=trp[:],
                                         lhsT=G[:, 0, k * P:(k + 1) * P],
                                         rhs=ident[:], is_transpose=True)
                        nc.scalar.activation(out=aggT[:, k, :], in_=trp[:],
                                             func=ACT.Copy)
                    for m in range(2):
                        h1p = psA.tile([P, P], f32, space="PSUM", tag="ps")
                        for k in range(2):
                            nc.tensor.matmul(
                                out=h1p[:],
                                lhsT=W1e[:, k, m * P:(m + 1) * P],
                                rhs=aggT[:, k, :],
                                start=(k == 0), stop=(k == 1))
                        nc.scalar.activation(
                            out=h1win[m][:, (t % GRP) * P:(t % GRP + 1) * P],
                            in_=h1p[:], func=ACT.Relu)
                    if t % GRP == GRP - 1:
                        g = t // GRP
                        for m in range(2):
                            seg = h1win[m][:].rearrange("p (b s) -> p b s",
                                                        s=S)
                            nc.vector.reduce_sum(
                                out=aggT2[m][:, g * 64:(g + 1) * 64],
                                in_=seg, axis=mybir.AxisListType.X)

            def dec_compute(q):
                qs = slice(q * QB, (q + 1) * QB)
                for (wt, dst) in ((W2e, muT), (W3e, lvT)):
                    mp = psA.tile([P, QB], f32, space="PSUM", tag="ps")
                    for k in range(2):
                        nc.tensor.matmul(out=mp[:], lhsT=wt[:, k, :],
                                         rhs=aggT2[k][:, qs],
                                         start=(k == 0), stop=(k == 1))
                    nc.scalar.activation(out=dst[:, qs], in_=mp[:],
                                         func=ACT.Copy)
                for (srcT, dst) in ((muT, o_mu), (lvT, o_lv)):
                    for j in range(2):
                        mm = 2 * q + j
                        trp = psA.tile([P, P], f32, space="PSUM", tag="ps")
                        nc.tensor.matmul(out=trp[:],
                                         lhsT=srcT[:, mm * P:(mm + 1) * P],
                                         rhs=ident[:], is_transpose=True)
                        osb = work.tile([P, P], f32, name="osb", tag="osb")
                        nc.scalar.activation(out=osb[:], in_=trp[:],
                                             func=ACT.Copy)
                        nc.sync.dma_start(
                            out=dst.ap()[mm * P:(mm + 1) * P, :], in_=osb[:])

                o1T = []
                for m in range(2):
                    xp = psA.tile([P, QB], f32, space="PSUM", tag="ps")
                    nc.tensor.matmul(out=xp[:],
                                     lhsT=Wd1s[:, m * P:(m + 1) * P],
                                     rhs=muT[:, qs], start=True, stop=True)
                    o1 = decp.tile([P, QB], f32, name=f"o1T{m}", tag="dec")
                    nc.scalar.activation(out=o1[:], in_=xp[:], func=ACT.Relu)
                    o1T.append(o1)
                o2T = []
                for m in range(2):
                    xp = psA.tile([P, QB], f32, space="PSUM", tag="ps")
                    for k in range(2):
                        nc.tensor.matmul(out=xp[:],
                                         lhsT=Wd2s[:, k, m * P:(m + 1) * P],
                                         rhs=o1T[k][:],
                                         start=(k == 0), stop=(k == 1))
                    o2 = decp.tile([P, QB], f32, name=f"o2T{m}", tag="dec")
                    nc.scalar.activation(out=o2[:], in_=xp[:], func=ACT.Relu)
                    o2T.append(o2)

                def emb(wt, nm):
                    tiles = []
                    for m in range(2):
                        xp = psA.tile([P, QB], f32, space="PSUM", tag="ps")
                        for k in range(2):
                            nc.tensor.matmul(
                                out=xp[:],
                                lhsT=wt[:, k, m * P:(m + 1) * P],
                                rhs=o2T[k][:],
                                start=(k == 0), stop=(k == 1))
                        e = decp.tile([P, QB], f32, name=f"{nm}{m}",
                                      tag="dec")
                        nc.scalar.activation(out=e[:], in_=xp[:],
                                             func=ACT.Copy)
                        tiles.append(e)
                    return tiles

                E1T = emb(Wm1s, "E1T")
                E2T = emb(Wm2s, "E2T")

                def colnorm_scale(Es, outs):
                    ssp = psS.tile([1, QB], f32, space="PSUM", tag="small")
                    sqs = []
                    for m in range(2):
                        sq = work.tile([P, QB], f32, name=f"sq{m}",
                                       tag=f"sq{m}")
                        nc.vector.tensor_mul(out=sq[:], in0=Es[m][:],
                                             in1=Es[m][:])
                        sqs.append(sq)
                    for m in range(2):
                        nc.tensor.matmul(out=ssp[:], lhsT=ones[:],
                                         rhs=sqs[m][:],
                                         start=(m == 0), stop=(m == 1))
                    r = work.tile([1, QB], f32, name="r", tag="r")
                    nc.scalar.activation(out=r[:], in_=ssp[:], func=ACT.Sqrt)
                    nc.vector.tensor_scalar_max(out=r[:], in0=r[:],
                                                scalar1=1e-12)
                    nc.vector.reciprocal(out=r[:], in_=r[:])
                    # broadcast r along partitions on the PE (K=1 matmul)
                    bc = psA.tile([P, QB], f32, space="PSUM", tag="ps")
                    nc.tensor.matmul(out=bc[:], lhsT=ones1[:], rhs=r[:],
                                     start=True, stop=True)
                    for m in range(2):
                        nc.vector.tensor_mul(out=outs[m], in0=Es[m][:],
                                             in1=bc[:])

                qs2 = slice(q * QB, (q + 1) * QB)
                e2h = [decp.tile([P, QB], f32, name=f"e2h{m}", tag="dec")
                       for m in range(2)]
                colnorm_scale(E1T, [e1nT[m][:, qs2] for m in range(2)])
                colnorm_scale(E2T, [e2h[m][:] for m in range(2)])
                for m in range(2):
                    nc.sync.dma_start(out=cc_in[q][m * P:(m + 1) * P, :],
                                      in_=e2h[m][:])

            def ag_and_w(q):
                nc.gpsimd.collective_compute(
                    "AllGather", mybir.AluOpType.bypass,
                    replica_groups=[list(range(NCORES))],
                    ins=[cc_in[q][:]], outs=[cc_out[q][:]])
                for r_ in range(NCORES):
                    for k in range(2):
                        Rt = persist.tile([P, QB], f32, name=f"R{r_}_{k}_{q}")
                        nc.sync.dma_start(
                            out=Rt[:],
                            in_=cc_out[q][r_ * 2 * P + k * P:
                                          r_ * 2 * P + (k + 1) * P, :])
                        R[r_][k][q] = Rt
                for k in range(2):
                    wacc = persist.tile([P, 1], f32, name=f"wacc{q}{k}")
                    for r_ in range(NCORES):
                        tt = work.tile([P, QB], f32, name="tt", tag="tt")
                        nc.vector.tensor_mul(
                            out=tt[:], in0=R[r_][k][q][:],
                            in1=vb[:, r_ * BLOC + q * QB:
                                   r_ * BLOC + (q + 1) * QB])
                        rs = work.tile([P, 1], f32, name="rs", tag="rs")
                        nc.vector.reduce_sum(out=rs[:], in_=tt[:],
                                             axis=mybir.AxisListType.X)
                        if r_ == 0:
                            nc.vector.tensor_copy(out=wacc[:], in_=rs[:])
                        else:
                            nc.vector.tensor_add(out=wacc[:], in0=wacc[:],
                                                 in1=rs[:])
                    w_part[q][k] = wacc

            def rec_pair(rr, qc):
                # rows: m-tile rr; cols: quarter qc of every rank
                for r_ in range(NCORES):
                    rp = psA.tile([P, QB], f32, space="PSUM", tag="ps")
                    for k in range(2):
                        nc.tensor.matmul(
                            out=rp[:],
                            lhsT=e1nT[k][:, rr * P:(rr + 1) * P],
                            rhs=R[r_][k][qc][:],
                            start=(k == 0), stop=(k == 1))
                    rsb = work.tile([P, QB], f32, name="rsb", tag="rsb")
                    nc.scalar.activation(out=rsb[:], in_=rp[:], func=ACT.Copy)
                    nc.sync.dma_start(
                        out=o_rec.ap()[rr * P:(rr + 1) * P,
                                       r_ * BLOC + qc * QB:
                                       r_ * BLOC + (qc + 1) * QB],
                        in_=rsb[:])

            # ---------- main schedule ----------
            # halves of seeds; AG(0) trigger sits 15 tiles into half 1 so
            # its inputs are ready when the gpsimd stream reaches it.
            emit_tiles(0, 10)
            emit_tiles(10, 20)
            dec_compute(0)
            emit_tiles(20, 35)
            ag_and_w(0)
            emit_tiles(35, 40)
            rec_pair(0, 0)
            rec_pair(1, 0)
            dec_compute(1)
            ag_and_w(1)
            for (rr, qc) in ((2, 0), (3, 0), (0, 1), (1, 1), (2, 1), (3, 1)):
                rec_pair(rr, qc)

            # ---------- gen = e1n @ w + c ----------
            w_k = []
            for k in range(2):
                wk = work.tile([P, 1], f32, name=f"wk{k}", tag="wk")
                nc.vector.tensor_add(out=wk[:], in0=w_part[0][k][:],
                                     in1=w_part[1][k][:])
                w_k.append(wk)
            for mm in range(4):
                gp = psS.tile([P, 1], f32, space="PSUM", tag="small")
                for k in range(2):
                    nc.tensor.matmul(out=gp[:],
                                     lhsT=e1nT[k][:, mm * P:(mm + 1) * P],
                                     rhs=w_k[k][:],
                                     start=(k == 0), stop=(k == 1))
                gsb = work.tile([P, 1], f32, name="gsb", tag="gsb")
                nc.vector.tensor_add(out=gsb[:], in0=gp[:], in1=cb[:])
                nc.sync.dma_start(out=o_gen.ap()[mm * P:(mm + 1) * P, :],
                                  in_=gsb[:])

    nc.compile()
    return nc


def _get_compiled():
    global _COMPILED
    if _COMPILED is None:
        _COMPILED = _build()
    return _COMPILED


def kernel(nodes, neighs2, sub_adj, feat_table, W1, W2, W3,
           Wd1, Wd2, Wm1, Wm2, Wg, bg, Wdisc):
    global LAST_EXEC_NS, LAST_RES
    from concourse.bass_utils import run_bass_kernel_spmd

    nodes = np.asarray(nodes).astype(np.int32)
    neighs2 = np.asarray(neighs2).astype(np.int32)
    sub_adj = np.ascontiguousarray(np.asarray(sub_adj, dtype=np.float32))
    feat_table = np.ascontiguousarray(np.asarray(feat_table, dtype=np.float32))
    ws = {}
    for nm, w in (("W1", W1), ("W2", W2), ("W3", W3), ("Wd1", Wd1),
                  ("Wd2", Wd2), ("Wm1", Wm1), ("Wm2", Wm2), ("Wg", Wg),
                  ("bg", bg), ("Wdisc", Wdisc)):
        ws[nm] = np.ascontiguousarray(np.asarray(w, dtype=np.float32))

    nc = _get_compiled()

    in_maps = []
    for c in range(NCORES):
        m = {
            "feat": feat_table,
            "nidx": np.ascontiguousarray(neighs2[c * NLOC:(c + 1) * NLOC, :]),
            "nodes": np.ascontiguousarray(nodes[c * BLOC:(c + 1) * BLOC]),
            "subadj": np.ascontiguousarray(sub_adj[c * BLOC:(c + 1) * BLOC, :]),
        }
        m.update(ws)
        in_maps.append(m)

    res = run_bass_kernel_spmd(nc, in_maps, core_ids=list(range(NCORES)),
                               trace=TRACE)
    LAST_EXEC_NS = res.exec_time_ns
    LAST_RES = res

    mu = np.concatenate([res.results[c]["mu"] for c in range(NCORES)], axis=0)
    lv = np.concatenate([res.results[c]["lv"] for c in range(NCORES)], axis=0)
    rec = np.concatenate([res.results[c]["rec"] for c in range(NCORES)],
                         axis=0)
    orig = np.concatenate([res.results[c]["orig"] for c in range(NCORES)],
                          axis=0)
    gen = np.concatenate([res.results[c]["gen"] for c in range(NCORES)],
                         axis=0)
    gan_pred = np.concatenate([orig, gen], axis=0)
    gan_label = np.concatenate(
        [np.ones_like(orig), np.zeros_like(gen)], axis=0)
    return (mu, lv, rec, gan_pred, gan_label)
